# revision 1
# baseline (speedup 1.0000x reference)
"""Trainium2 Bass kernel for a Mamba-1-style MixerBlock.

Reference computation (shapes: X[2,1024,1024], D=2048, N=16, K=4):
  Xn = LayerNorm(X) * g + b
  X_main = silu(conv_b + causal_depthwise_conv1d(Xn @ W_up1.T))
  pp = X_main @ W_ll.T + b_ll ; delta = softplus(pp[:, :D]); Bm, Cm = pp[:, D:D+N], pp[:, D+N:]
  a = exp(delta * A)  (A = -exp(A_log), [D,N])
  u = (a-1)/A * Bm * X_main        (per (b,l,d,n))
  h[t] = a[t] h[t-1] + u[t]        (scan over L per (b,d,n))
  y_ssm[t,d] = sum_n Cm[t,n] h[t,d,n]
  out = X + (y_ssm * silu(Xn @ W_up2.T)) @ W_down.T + b_down

Sharding: sequence-parallel over 8 cores (2 batches x 4 L-quarters of 256).
Each core redundantly recomputes a 64-step scan warmup (decays are fast:
contributions older than 64 steps are < 1e-9 relative), so the kernel is
embarrassingly parallel - no collectives.

Per-core layout: channels on partitions, sequence on the free dim.
The SSM middle runs per 128-channel d-tile in [128, (n, l)] tiles; the L-scan
is a native DVE tensor_tensor_scan chaining the 8 n-segments per instruction
(the decay at each segment start is zeroed, which exactly encodes h=-u start).
"""

import functools
import numpy as np

D_OUTER, D, N, K = 1024, 2048, 16, 4
B_SZ, L = 2, 1024
NCORES = 8
LO = 256            # own sequence steps per core
WARM = 48           # redundant scan warmup steps (worst-case leak ~2e-11)
LW = WARM + LO      # 320: domain of X_main/delta/scan
LC = LW + K         # 324: LayerNorm/mm1 domain (conv taps + fp32r even pad)
NT_D = D // 128     # 16 d-tiles
NT_K = D_OUTER // 128  # 8 k-tiles over d_outer
last_result = None
USE_F32R = False     # fp32r matmuls: 4x PE speed, ~1.5e-4 matmul rel err
NHALF = 4           # n-values per group (SBUF pressure: process n in 4 groups)


@functools.lru_cache(maxsize=2)
def _build_program(phases: str = "0ABCD"):
    import os
    import concourse.bass as bass
    import concourse.bacc as bacc
    import concourse.mybir as mybir
    import concourse.tile as tile
    from concourse.masks import make_identity

    f32 = mybir.dt.float32
    f32r = mybir.dt.float32r if USE_F32R else mybir.dt.float32
    AF = mybir.ActivationFunctionType
    OP = mybir.AluOpType

    # Steer the act-table-load pass: keep Exp and Ln only in their shared
    # set so phase C needs a single table load (ids/order preserved).
    import concourse.hw_specs as hw_specs
    if not getattr(bacc, "_act_tables_patched", False):
        _orig_gat = hw_specs.get_activation_tables

        def _gat(module_arch):
            tabs = _orig_gat(module_arch)
            AT = mybir.ActivationFunctionType
            for name, fns in tabs.items():
                if name != "natural_log_exp_and_others":
                    fns.discard(AT.Exp)
                    fns.discard(AT.Ln)
            return tabs

        bacc.get_activation_tables = _gat
        bacc._act_tables_patched = True

    nc = bacc.Bacc("TRN2", target_bir_lowering=False)

    # ---- DRAM I/O ----
    Xs_d = nc.dram_tensor("Xs", [LC, D_OUTER], f32, kind="ExternalInput")
    W1T_d = nc.dram_tensor("W1T", [D_OUTER, D], f32r, kind="ExternalInput")
    W2T_d = nc.dram_tensor("W2T", [D_OUTER, D], f32r, kind="ExternalInput")
    WllT_d = nc.dram_tensor("WllT", [D, 2 * N + D], f32r, kind="ExternalInput")
    WdT_d = nc.dram_tensor("WdT", [D, D_OUTER], f32, kind="ExternalInput")
    convw_d = nc.dram_tensor("convw", [D, K], f32, kind="ExternalInput")
    cb2_d = nc.dram_tensor("cb2", [D, 1], f32, kind="ExternalInput")
    bd_d = nc.dram_tensor("bd", [D, 1], f32, kind="ExternalInput")
    bbc_d = nc.dram_tensor("bbc", [2 * N, 1], f32, kind="ExternalInput")
    c2_d = nc.dram_tensor("c2", [D, 1], f32, kind="ExternalInput")
    bdown_d = nc.dram_tensor("bdown", [D_OUTER, 1], f32, kind="ExternalInput")
    A_d = nc.dram_tensor("A", [D, N], f32, kind="ExternalInput")
    invAv_d = nc.dram_tensor("invAv", [2 * N, 1], f32, kind="ExternalInput")
    mask_d = nc.dram_tensor("mask", [1, LW], f32, kind="ExternalInput")
    Y_d = nc.dram_tensor("Y", [D_OUTER, LO], f32, kind="ExternalOutput")

    def bcast_n(t, nrep):
        # stride-0 broadcast of a [128, F] tile to [128, nrep, F]
        return bass.AP(tensor=t.tensor, offset=t.offset,
                       ap=[t.ap[0], [0, nrep], t.ap[1]])

    with tile.TileContext(nc) as tc:
        with (
            tc.tile_pool(name="const", bufs=1) as const,
            tc.tile_pool(name="persist", bufs=1) as persist,
            tc.tile_pool(name="work", bufs=2) as work,
            tc.tile_pool(name="big", bufs=2) as big,
            tc.tile_pool(name="bigwu", bufs=3) as bigwu,
            tc.tile_pool(name="wstream", bufs=2) as wstream,
            tc.tile_pool(name="psT", bufs=2, space="PSUM") as psT,
            tc.tile_pool(name="psA", bufs=6, space="PSUM") as psA,
        ):
            # ---- constants ----
            ident = const.tile([128, 128], f32, tag="ident")
            make_identity(nc, ident)
            eps_sb = const.tile([128, 1], f32, tag="eps")
            nc.vector.memset(eps_sb, 1e-5)

            convw_sb, cb2_sb, bd_sb, c2_sb, A_sb = [], [], [], [], []
            for dt in range(NT_D):
                r = slice(dt * 128, (dt + 1) * 128)
                t = const.tile([128, K], f32, tag=f"cw{dt}")
                nc.sync.dma_start(out=t, in_=convw_d[r, :]); convw_sb.append(t)
                t = const.tile([128, 1], f32, tag=f"cb{dt}")
                nc.sync.dma_start(out=t, in_=cb2_d[r, :]); cb2_sb.append(t)
                t = const.tile([128, 1], f32, tag=f"bd{dt}")
                nc.sync.dma_start(out=t, in_=bd_d[r, :]); bd_sb.append(t)
                t = const.tile([128, 1], f32, tag=f"c2{dt}")
                nc.sync.dma_start(out=t, in_=c2_d[r, :]); c2_sb.append(t)
                t = const.tile([128, N], f32, tag=f"A{dt}")
                nc.sync.dma_start(out=t, in_=A_d[r, :]); A_sb.append(t)
            bbc_sb = const.tile([2 * N, 1], f32, tag="bbc")
            nc.sync.dma_start(out=bbc_sb, in_=bbc_d[:, :])
            invAv_sb = const.tile([2 * N, 1], f32, tag="invAv")
            nc.sync.dma_start(out=invAv_sb, in_=invAv_d[:, :])
            mask_sb = const.tile([2 * N, LW], f32, tag="mask")
            m_ap = mask_d[:, :]
            nc.sync.dma_start(
                out=mask_sb,
                in_=bass.AP(tensor=m_ap.tensor, offset=m_ap.offset,
                            ap=[[0, 2 * N], m_ap.ap[1]]))
            bdown_sb = []
            for e8 in range(NT_K):
                t = const.tile([128, 1], f32, tag=f"bdn{e8}")
                nc.sync.dma_start(out=t, in_=bdown_d[e8 * 128:(e8 + 1) * 128, :])
                bdown_sb.append(t)

            # ---- Phase 0: load X rows, LayerNorm, transposes ----
            rows = [128, 128, LC - 256]
            p0_cm = tc.tile_pool(name="p0", bufs=1)
            p0 = p0_cm.__enter__()
            xhat_rows, mus, sigs = [], [], []
            for i in range(3):
                r = rows[i]
                xr = p0.tile([128, D_OUTER], f32, tag=f"xr{i}")
                nc.sync.dma_start(out=xr[:r, :],
                                  in_=Xs_d[i * 128:i * 128 + r, :])
                # bn_stats free-dim max is 512: two subgroups then aggregate
                stats = work.tile([128, 2, 6], f32, tag="stats")
                for sg in range(2):
                    nc.vector.bn_stats(out=stats[:r, sg, :],
                                       in_=xr[:r, sg * 512:(sg + 1) * 512])
                mv = work.tile([128, 2], f32, tag="mv")
                nc.vector.bn_aggr(out=mv[:r, :], in_=stats[:r, :, :])
                sig = work.tile([128, 1], f32, tag=f"sig{i}")
                nc.scalar.activation(out=sig[:r], in_=mv[:r, 1:2],
                                     func=AF.Sqrt, bias=eps_sb[:r, 0:1],
                                     scale=1.0)
                rsig = work.tile([128, 1], f32, tag=f"rsig{i}")
                nc.vector.reciprocal(out=rsig[:r], in_=sig[:r])
                nmu = work.tile([128, 1], f32, tag="nmu")
                nc.vector.tensor_scalar(out=nmu[:r], in0=mv[:r, 0:1],
                                        scalar1=rsig[:r, 0:1], scalar2=-1.0,
                                        op0=OP.mult, op1=OP.mult)
                mu = work.tile([128, 1], f32, tag=f"mu{i}")
                nc.vector.tensor_copy(out=mu[:r], in_=mv[:r, 0:1])
                xh = p0.tile([128, D_OUTER], f32, tag=f"xh{i}")
                nc.vector.tensor_scalar(out=xh[:r, :], in0=xr[:r, :],
                                        scalar1=rsig[:r, 0:1],
                                        scalar2=nmu[:r, 0:1],
                                        op0=OP.mult, op1=OP.add)
                xhat_rows.append(xh)
                mus.append(mu)
                sigs.append(sig)

            # stage mu/sig to DRAM, read back broadcast over partitions
            # (for reconstructing X^T for the residual: X = xhat*sig + mu)
            with tc.tile_pool(name="dres", bufs=1, space="DRAM") as drp:
                mu_d = drp.tile([3 * 128, 1], f32, tag="mu_d")
                sig_d = drp.tile([3 * 128, 1], f32, tag="sig_d")
                for i in range(3):
                    r = rows[i]
                    nc.sync.dma_start(out=mu_d[i * 128:i * 128 + r, :],
                                      in_=mus[i][:r])
                    nc.sync.dma_start(out=sig_d[i * 128:i * 128 + r, :],
                                      in_=sigs[i][:r])
                mu_bc = persist.tile([128, LO], f32, tag="mu_bc")
                sig_bc = persist.tile([128, LO], f32, tag="sig_bc")
                own0 = WARM + K - 1
                for (dst, srcd) in ((mu_bc, mu_d), (sig_bc, sig_d)):
                    s_ap = srcd[own0:own0 + LO, :]
                    nc.sync.dma_start(
                        out=dst,
                        in_=bass.AP(tensor=s_ap.tensor, offset=s_ap.offset,
                                    ap=[[0, 128], [1, LO]]))

            xhatT = []
            for kt in range(NT_K):
                xt = persist.tile([128, LC], f32r, tag=f"xhT{kt}")
                cs = slice(kt * 128, (kt + 1) * 128)
                for i in range(3):
                    r = rows[i]
                    pt = psT.tile([128, 128], f32, tag="tp")
                    nc.tensor.transpose(pt[:, :r], xhat_rows[i][:r, cs],
                                        ident[:r, :r])
                    nc.scalar.copy(out=xt[:, i * 128:i * 128 + r],
                                   in_=pt[:, :r])
                xhatT.append(xt)
            p0_cm.__exit__(None, None, None)

            # ---- Phase A: mm1 + causal depthwise conv + silu -> X_main ----
            X_main = []
            for dt in range(NT_D if "A" in phases else 0):
                w1t = wstream.tile([128, NT_K, 128], f32r, tag="wst")
                nc.sync.dma_start(
                    out=w1t,
                    in_=W1T_d.rearrange("(kt p) m -> p kt m", p=128)
                        [:, :, dt * 128:(dt + 1) * 128])
                ps = psA.tile([128, LC], f32, tag="mm")
                for kt in range(NT_K):
                    nc.tensor.matmul(ps, w1t[:, kt, :],
                                     xhatT[kt],
                                     start=(kt == 0), stop=(kt == NT_K - 1))
                acc = None
                for tap in range(K):
                    nxt = work.tile([128, LW], f32, tag="cacc")
                    if acc is None:
                        nc.vector.tensor_scalar(
                            out=nxt, in0=ps[:, tap:tap + LW],
                            scalar1=convw_sb[dt][:, tap:tap + 1], scalar2=None,
                            op0=OP.mult)
                    else:
                        nc.vector.scalar_tensor_tensor(
                            out=nxt, in0=ps[:, tap:tap + LW],
                            scalar=convw_sb[dt][:, tap:tap + 1], in1=acc,
                            op0=OP.mult, op1=OP.add)
                    acc = nxt
                xm = persist.tile([128, LW], f32r, tag=f"xm{dt}")
                nc.scalar.activation(out=xm, in_=acc, func=AF.Silu,
                                     bias=cb2_sb[dt][:, 0:1], scale=1.0)
                X_main.append(xm)

            # ---- Phase A2: gate = silu(xhat @ W2) (own L only) ----
            X_gate = []
            gate_silus = []
            for dt in range(NT_D if "A" in phases else 0):
                w2t = wstream.tile([128, NT_K, 128], f32r, tag="wst")
                nc.sync.dma_start(
                    out=w2t,
                    in_=W2T_d.rearrange("(kt p) m -> p kt m", p=128)
                        [:, :, dt * 128:(dt + 1) * 128])
                ps = psA.tile([128, LO], f32, tag="mm")
                for kt in range(NT_K):
                    nc.tensor.matmul(ps, w2t[:, kt, :],
                                     xhatT[kt][:, WARM + K - 1:WARM + K - 1 + LO],
                                     start=(kt == 0), stop=(kt == NT_K - 1))
                xg = persist.tile([128, LO], f32, tag=f"xg{dt}")
                si = nc.scalar.activation(out=xg, in_=ps, func=AF.Silu,
                                          bias=c2_sb[dt][:, 0:1], scale=1.0)
                gate_silus.append(si)
                X_gate.append(xg)

            # ---- Phase B: B/C rows of pp + partition-broadcast ----
            wbt = wstream.tile([128, NT_D, 2 * N], f32r, tag="wst")
            nc.sync.dma_start(
                out=wbt,
                in_=WllT_d.rearrange("(kt p) e -> p kt e", p=128)[:, :, D:])
            psbc = psA.tile([2 * N, LW], f32, tag="mm")
            for kt in range(NT_D):
                nc.tensor.matmul(psbc, wbt[:, kt, :],
                                 X_main[kt],
                                 start=(kt == 0), stop=(kt == NT_D - 1))
            bc_raw = work.tile([2 * N, LW], f32, tag="bcraw")
            nc.scalar.activation(out=bc_raw, in_=psbc, func=AF.Identity,
                                 bias=bbc_sb[:, 0:1], scale=1.0)
            bci = work.tile([2 * N, LW], f32, tag="bci")
            nc.vector.scalar_tensor_tensor(out=bci, in0=bc_raw,
                                           scalar=invAv_sb[:, 0:1],
                                           in1=mask_sb, op0=OP.mult,
                                           op1=OP.mult)
            Bm_bcI = persist.tile([128, N, LW], f32, tag="BmbcI")
            Cm_bc = persist.tile([128, N, LO], f32, tag="Cmbc")
            with tc.tile_pool(name="dstage", bufs=1, space="DRAM") as dpool:
                bci_dram = dpool.tile([2 * N, LW], f32, tag="bcid")
                nc.sync.dma_start(out=bci_dram, in_=bci)
                for n in range(N):
                    src_b = bci_dram[n:n + 1, :]
                    nc.sync.dma_start(
                        out=Bm_bcI[:, n, :],
                        in_=bass.AP(tensor=src_b.tensor, offset=src_b.offset,
                                    ap=[[0, 128]] + src_b.ap[1:]))
                    src_c = bci_dram[N + n:N + n + 1, WARM:LW]
                    nc.sync.dma_start(
                        out=Cm_bc[:, n, :],
                        in_=bass.AP(tensor=src_c.tensor, offset=src_c.offset,
                                    ap=[[0, 128]] + src_c.ap[1:]))

            # ---- Phase C: per d-tile: delta, a, u, scan, y ----
            y_gated = []
            for dt in range(NT_D):
                wllt = wstream.tile([128, NT_D, 128], f32r, tag="wst")
                nc.sync.dma_start(
                    out=wllt,
                    in_=WllT_d.rearrange("(kt p) e -> p kt e", p=128)
                        [:, :, dt * 128:(dt + 1) * 128])
                ps = psA.tile([128, LW], f32, tag="mm")
                for kt in range(NT_D):
                    nc.tensor.matmul(ps, wllt[:, kt, :],
                                     X_main[kt],
                                     start=(kt == 0), stop=(kt == NT_D - 1))
                # softplus(x) = ln(exp(x) + 1); exp & ln share one ACT table set
                e1 = work.tile([128, LW], f32, tag="e1")
                e1i = nc.scalar.activation(out=e1, in_=ps, func=AF.Exp,
                                           bias=bd_sb[dt][:, 0:1], scale=1.0)
                if dt == 0:
                    from concourse.tile_rust import add_dep_helper
                    for si in gate_silus:
                        add_dep_helper(e1i.ins, si.ins, False,
                                       "ACT table-set phase ordering")
                delta = work.tile([128, LW], f32, tag="delta")
                nc.scalar.activation(out=delta, in_=e1, func=AF.Ln,
                                     bias=1.0, scale=1.0)

                y_parts = []
                for hf in range(N // NHALF):
                    ns = slice(hf * NHALF, (hf + 1) * NHALF)
                    a_t = big.tile([128, NHALF, LW], f32, tag="a")
                    for i in range(NHALF):
                        n = hf * NHALF + i
                        nc.scalar.activation(out=a_t[:, i, :], in_=delta,
                                             func=AF.Exp, bias=0.0,
                                             scale=A_sb[dt][:, n:n + 1])
                    w_t = bigwu.tile([128, NHALF, LW], f32, tag="wu")
                    w_eng = nc.gpsimd if dt % 5 == 0 else nc.vector
                    w_eng.tensor_tensor(
                        out=w_t, in0=bcast_n(X_main[dt].bitcast(f32), NHALF),
                        in1=Bm_bcI[:, ns, :], op=OP.mult)
                    u_t = bigwu.tile([128, NHALF, LW], f32, tag="wu")
                    nc.vector.scalar_tensor_tensor(
                        out=u_t, in0=a_t, scalar=-1.0, in1=w_t,
                        op0=OP.add, op1=OP.mult)
                    # zero decay at each n-segment start: encodes h(start)=u
                    nc.vector.memset(a_t[:, :, 0:1], 0.0)
                    h_t = big.tile([128, NHALF, LW], f32, tag="h")
                    nc.vector.tensor_tensor_scan(
                        out=h_t.rearrange("p n l -> p (n l)"),
                        data0=a_t.rearrange("p n l -> p (n l)"),
                        data1=u_t.rearrange("p n l -> p (n l)"),
                        initial=0.0, op0=OP.mult, op1=OP.add)
                    hci = bigwu.tile([128, NHALF, LO], f32, tag="wu")
                    hc_eng = nc.vector if dt % 4 == 0 else nc.gpsimd
                    hc_eng.tensor_tensor(
                        out=hci,
                        in0=h_t[:, :, WARM:LW], in1=Cm_bc[:, ns, :],
                        op=OP.mult)
                    # sum over n: 2-level pairwise tree on Pool, all APs
                    # contiguous (frees DVE, no strided Q7 addressing)
                    yt = work.tile([128, 2, LO], f32, tag="yt")
                    nc.gpsimd.tensor_tensor(out=yt, in0=hci[:, 0:2, :],
                                            in1=hci[:, 2:4, :], op=OP.add)
                    y_h = work.tile([128, LO], f32, tag="yh")
                    nc.gpsimd.tensor_tensor(out=y_h, in0=yt[:, 0, :],
                                            in1=yt[:, 1, :], op=OP.add)
                    y_parts.append(y_h)
                ys0 = work.tile([128, LO], f32, tag="ysum")
                nc.gpsimd.tensor_tensor(out=ys0, in0=y_parts[0],
                                        in1=y_parts[1], op=OP.add)
                ys1 = work.tile([128, LO], f32, tag="ysum2")
                nc.gpsimd.tensor_tensor(out=ys1, in0=y_parts[2],
                                        in1=y_parts[3], op=OP.add)
                ysum = work.tile([128, LO], f32, tag="ysum3")
                nc.gpsimd.tensor_tensor(out=ysum, in0=ys0, in1=ys1, op=OP.add)
                yg = persist.tile([128, LO], f32, tag=f"yg{dt}")
                nc.vector.tensor_tensor(out=yg, in0=ysum, in1=X_gate[dt],
                                        op=OP.mult)
                y_gated.append(yg)

            # ---- Phase D: down projection + residual ----
            for e8 in range(NT_K):
                wdt = wstream.tile([128, NT_D, 128], f32, tag="wst")
                nc.sync.dma_start(
                    out=wdt,
                    in_=WdT_d.rearrange("(kt p) m -> p kt m", p=128)
                        [:, :, e8 * 128:(e8 + 1) * 128])
                ps = psA.tile([128, LO], f32, tag="mm")
                for dt in range(NT_D):
                    nc.tensor.matmul(ps, wdt[:, dt, :],
                                     y_gated[dt],
                                     start=(dt == 0), stop=(dt == NT_D - 1))
                xrec = work.tile([128, LO], f32, tag="xrec")
                nc.gpsimd.tensor_tensor(out=xrec,
                                        in0=xhatT[e8].bitcast(f32)
                                        [:, WARM + K - 1:WARM + K - 1 + LO],
                                        in1=sig_bc, op=OP.mult)
                xrec2 = work.tile([128, LO], f32, tag="xrec2")
                nc.vector.tensor_tensor(out=xrec2, in0=xrec, in1=mu_bc,
                                        op=OP.add)
                osb = work.tile([128, LO], f32, tag="osb")
                nc.vector.scalar_tensor_tensor(
                    out=osb, in0=ps, scalar=bdown_sb[e8][:, 0:1],
                    in1=xrec2, op0=OP.add, op1=OP.add)
                nc.sync.dma_start(out=Y_d[e8 * 128:(e8 + 1) * 128, :], in_=osb)

    nc.compile()
    return nc


def kernel(X, ln_g, ln_b, W_up1, conv_w, conv_b, W_ll, b_ll, A_log, W_up2,
           W_down, b_down):
    from concourse.bass_utils import run_bass_kernel_spmd

    f = np.float32
    X = np.asarray(X, f)
    A = -np.exp(np.asarray(A_log, f))
    assert np.allclose(A, A[0:1, :]), "kernel assumes A rows identical"
    c1 = (np.asarray(W_up1, f) @ np.asarray(ln_b, f)).astype(f)
    c2 = (np.asarray(W_up2, f) @ np.asarray(ln_b, f)).astype(f)
    cw = np.asarray(conv_w, f)[:, 0, :]                      # [D, K]
    cb2 = (np.asarray(conv_b, f) + c1 * cw.sum(1)).astype(f)
    shared = {
        "W1T": np.ascontiguousarray((np.asarray(W_up1, f)
                                     * np.asarray(ln_g, f)[None, :]).T),
        "W2T": np.ascontiguousarray((np.asarray(W_up2, f)
                                     * np.asarray(ln_g, f)[None, :]).T),
        "WllT": np.ascontiguousarray(np.asarray(W_ll, f).T),
        "WdT": np.ascontiguousarray(np.asarray(W_down, f).T),
        "convw": np.ascontiguousarray(cw),
        "cb2": cb2[:, None],
        "bd": np.asarray(b_ll, f)[:D, None],
        "bbc": np.asarray(b_ll, f)[D:, None],
        "c2": c2[:, None],
        "bdown": np.asarray(b_down, f)[:, None],
        "A": np.ascontiguousarray(A),
        "invAv": np.concatenate([1.0 / A[0], np.ones(N, f)]).astype(f)[:, None],
    }
    in_maps = []
    for c in range(NCORES):
        b, q = divmod(c, 4)
        l0 = q * LO
        lo_ext = l0 - (WARM + K - 1)
        xs = np.zeros((LC, D_OUTER), f)
        src0 = max(0, lo_ext)
        hi = min(l0 + LO + 1, L)
        xs[src0 - lo_ext:src0 - lo_ext + (hi - src0), :] = X[b, src0:hi, :]
        mask = np.ones((1, LW), f)
        if q == 0:
            mask[0, :WARM] = 0.0
        in_maps.append({"Xs": xs, "mask": mask, **shared})

    nc = _build_program()
    res = run_bass_kernel_spmd(nc, in_maps, core_ids=list(range(NCORES)))
    global last_result
    last_result = res

    out = np.empty((B_SZ, L, D_OUTER), f)
    for c in range(NCORES):
        b, q = divmod(c, 4)
        out[b, q * LO:(q + 1) * LO, :] = res.results[c]["Y"].T
    return out



# revision 32
# speedup vs baseline: 2.3358x; 2.3358x over previous
"""Trainium2 Bass kernel for a Mamba-1-style MixerBlock.

Reference computation (shapes: X[2,1024,1024], D=2048, N=16, K=4):
  Xn = LayerNorm(X) * g + b
  X_main = silu(conv_b + causal_depthwise_conv1d(Xn @ W_up1.T))
  pp = X_main @ W_ll.T + b_ll ; delta = softplus(pp[:, :D]); Bm, Cm = pp[:, D:D+N], pp[:, D+N:]
  a_n = exp(delta * A_n)  (A_n = -(n+1), shared across d)
  u = (a-1) * Bm/A * X_main        (per (b,l,d,n))
  h[t] = a[t] h[t-1] + u[t]        (scan over L per (b,d,n))
  y_ssm[t,d] = sum_n Cm[t,n] h[t,d,n]
  out = X + (y_ssm * silu(Xn @ W_up2.T)) @ W_down.T + b_down

Sharding: sequence-parallel over 8 cores (2 batches x 4 L-quarters of 256).
Each core redundantly recomputes a short scan warmup (decays are fast), so
the kernel is embarrassingly parallel - no collectives.

Implementation notes (cost-model driven):
  - All matmuls run in bf16 (1 PE cycle/row vs 4 for fp32); weights are
    pre-transposed and pre-laid-out on the host so every weight DMA is
    >=512B-contiguous per partition (full DMA bus efficiency).
  - a_n = E^(n+1) with E = exp(-delta): 12 powers from ACT directly,
    the last 4 from one bf16 DVE multiply (E^{13..16} = E^{5..8} * E^8).
  - u = (a-1)*w with am1 = a-1 on the DVE 4x tensor_scalar path and
    w/u/hci/y-tree as bf16 tensor_tensor (DVE 2x mode).
  - The L-scan is a native tensor_tensor_scan chaining the 16 n-segments
    per d-tile (decay zeroed at segment starts); scans and the depthwise
    conv run on Pool (scalar_tensor_tensor form, 0.6 eff) to keep DVE free
    for the bf16 2x work.
"""

import functools
import numpy as np

D_OUTER, D, N, K = 1024, 2048, 16, 4
B_SZ, L = 2, 1024
NCORES = 8
LO = 256            # own sequence steps per core
WARM = 16           # redundant scan warmup steps
LW = WARM + LO      # domain of X_main/delta/scan
LC = LW + K         # LayerNorm/mm1 domain (conv taps + even pad)
NT_D = D // 128     # 16 d-tiles
NT_K = D_OUTER // 128  # 8 k-tiles over d_outer
N_ACT_EXP = 16      # decay powers computed directly on ACT (rest via DVE)
last_result = None


@functools.lru_cache(maxsize=2)
def _build_program(phases: str = "0ABCD"):
    import concourse.bass as bass
    import concourse.bacc as bacc
    import concourse.mybir as mybir
    import concourse.tile as tile
    from concourse.masks import make_identity

    f32 = mybir.dt.float32
    bf16 = mybir.dt.bfloat16
    AF = mybir.ActivationFunctionType
    OP = mybir.AluOpType

    # Steer the act-table-load pass: keep Exp and Ln only in their shared
    # set so phase C needs a single table load (ids/order preserved).
    import concourse.hw_specs as hw_specs
    if not getattr(bacc, "_act_tables_patched", False):
        _orig_gat = hw_specs.get_activation_tables

        def _gat(module_arch):
            tabs = _orig_gat(module_arch)
            AT = mybir.ActivationFunctionType
            for name, fns in tabs.items():
                if name != "natural_log_exp_and_others":
                    fns.discard(AT.Exp)
                    fns.discard(AT.Ln)
            return tabs

        bacc.get_activation_tables = _gat
        bacc._act_tables_patched = True

    nc = bacc.Bacc("TRN2", target_bir_lowering=False)

    # ---- DRAM I/O ----
    # Weights arrive pre-arranged so each DMA slice is contiguous per
    # partition (see kernel() for the exact host-side layouts).
    Xs_d = nc.dram_tensor("Xs", [LC, D_OUTER], f32, kind="ExternalInput")
    W12R_d = nc.dram_tensor("W12R", [128, NT_D, 2, NT_K * 128], bf16,
                            kind="ExternalInput")
    WllR_d = nc.dram_tensor("WllR", [128, NT_D, NT_D * 128], bf16,
                            kind="ExternalInput")
    WbcR_d = nc.dram_tensor("WbcR", [128, NT_D * 2 * N], bf16,
                            kind="ExternalInput")
    WdR_d = nc.dram_tensor("WdR", [128, NT_D, NT_K * 128], bf16,
                           kind="ExternalInput")
    # packed per-d-tile constants: [..., 0:4]=convw taps, 4=cb2, 5=bd, 6=c2
    CstD_d = nc.dram_tensor("CstD", [128, NT_D, 7], f32, kind="ExternalInput")
    # packed misc: [:, 0:8]=bdown by e8, [:, 8:8+N]=A row (replicated)
    CstO_d = nc.dram_tensor("CstO", [128, 8 + N], f32, kind="ExternalInput")
    # [2N, 0]=bbc bias, [2N, 1]=invAv
    BbcI_d = nc.dram_tensor("BbcI", [2 * N, 2], f32, kind="ExternalInput")
    mask_d = nc.dram_tensor("mask", [1, LW], f32, kind="ExternalInput")
    Y_d = nc.dram_tensor("Y", [D_OUTER, LO], f32, kind="ExternalOutput")

    def bcast_n(t, nrep):
        # stride-0 broadcast of a [128, F] tile to [128, nrep, F]
        return bass.AP(tensor=t.tensor, offset=t.offset,
                       ap=[t.ap[0], [0, nrep], t.ap[1]])

    def pool_mul(eng, out, in0, in1):
        # tensor-tensor multiply in scalar_tensor_tensor form: on Pool this
        # hits the 0.6-efficiency TensorScalarPtr path instead of the
        # 0.42-efficiency TensorTensor path.
        return eng.scalar_tensor_tensor(out=out, in0=in0, scalar=1.0,
                                        in1=in1, op0=OP.mult, op1=OP.mult)

    with tile.TileContext(nc) as tc:
        with (
            tc.tile_pool(name="const", bufs=1) as const,
            tc.tile_pool(name="persist", bufs=1) as persist,
            tc.tile_pool(name="work", bufs=2) as work,
            tc.tile_pool(name="nl", bufs=2) as nl,
            tc.tile_pool(name="nl4", bufs=2) as nl4,
            tc.tile_pool(name="nl3", bufs=3) as nl3,
            tc.tile_pool(name="wstream", bufs=2) as wstream,
            tc.tile_pool(name="psT", bufs=1, space="PSUM") as psT,
            tc.tile_pool(name="psA", bufs=2, space="PSUM") as psA,
            tc.tile_pool(name="psY", bufs=1, space="PSUM") as psY,
            tc.tile_pool(name="psD", bufs=1, space="PSUM") as psD,
        ):
            # ---- constants (batched into 4 DMAs) ----
            ident = const.tile([128, 128], f32, tag="ident")
            make_identity(nc, ident)
            identb = const.tile([128, 128], bf16, tag="identb")
            nc.vector.tensor_copy(out=identb, in_=ident)
            eps_sb = const.tile([128, 1], f32, tag="eps")
            nc.vector.memset(eps_sb, 1e-5)

            cstD = const.tile([128, NT_D, 7], f32, tag="cstD")
            nc.sync.dma_start(out=cstD, in_=CstD_d[:, :, :])
            cstO = const.tile([128, 8 + N], f32, tag="cstO")
            nc.sync.dma_start(out=cstO, in_=CstO_d[:, :])
            bbcinv = const.tile([2 * N, 2], f32, tag="bbcinv")
            nc.sync.dma_start(out=bbcinv, in_=BbcI_d[:, :])
            mask_sb = const.tile([2 * N, LW], f32, tag="mask")
            m_ap = mask_d[:, :]
            nc.sync.dma_start(
                out=mask_sb,
                in_=bass.AP(tensor=m_ap.tensor, offset=m_ap.offset,
                            ap=[[0, 2 * N], m_ap.ap[1]]))
            convw_sb = [cstD[:, dt, 0:4] for dt in range(NT_D)]
            cb2_sb = [cstD[:, dt, 4:5] for dt in range(NT_D)]
            bd_sb = [cstD[:, dt, 5:6] for dt in range(NT_D)]
            c2_sb = [cstD[:, dt, 6:7] for dt in range(NT_D)]
            bdown_sb = [cstO[:, e8:e8 + 1] for e8 in range(NT_K)]
            A_sb = cstO[:, 8:8 + N]
            bbc_sb = bbcinv[:, 0:1]
            invAv_sb = bbcinv[:, 1:2]

            # ---- Phase 0: load X rows, LayerNorm, transposes ----
            rows = [128, 128, LC - 256]
            p0_cm = tc.tile_pool(name="p0", bufs=2)
            p0 = p0_cm.__enter__()
            p0x_cm = tc.tile_pool(name="p0x", bufs=2)
            p0x = p0x_cm.__enter__()
            xhat_rows, mus, sigs = [], [], []
            xhatT = []
            for kt in range(NT_K):
                xt = persist.tile([128, LC], bf16, tag=f"xhT{kt}")
                xhatT.append(xt)
            for i in range(3):
                r = rows[i]
                xr = p0x.tile([128, D_OUTER], f32, tag="xr")
                nc.sync.dma_start(out=xr[:r, :],
                                  in_=Xs_d[i * 128:i * 128 + r, :])
                # bn_stats free-dim max is 512: two subgroups then aggregate
                stats = work.tile([128, 2, 6], f32, tag="stats")
                for sg in range(2):
                    nc.vector.bn_stats(out=stats[:r, sg, :],
                                       in_=xr[:r, sg * 512:(sg + 1) * 512])
                mv = work.tile([128, 2], f32, tag="mv")
                nc.vector.bn_aggr(out=mv[:r, :], in_=stats[:r, :, :])
                sig = work.tile([128, 1], f32, tag=f"sig{i}")
                nc.scalar.activation(out=sig[:r], in_=mv[:r, 1:2],
                                     func=AF.Sqrt, bias=eps_sb[:r, 0:1],
                                     scale=1.0)
                rsig = work.tile([128, 1], f32, tag=f"rsig{i}")
                nc.vector.reciprocal(out=rsig[:r], in_=sig[:r])
                nmu = work.tile([128, 1], f32, tag="nmu")
                nc.vector.tensor_scalar(out=nmu[:r], in0=mv[:r, 0:1],
                                        scalar1=rsig[:r, 0:1], scalar2=-1.0,
                                        op0=OP.mult, op1=OP.mult)
                mu = work.tile([128, 1], f32, tag=f"mu{i}")
                nc.vector.tensor_copy(out=mu[:r], in_=mv[:r, 0:1])
                xh = p0.tile([128, D_OUTER], f32, tag="xh")
                nc.vector.tensor_scalar(out=xh[:r, :], in0=xr[:r, :],
                                        scalar1=rsig[:r, 0:1],
                                        scalar2=nmu[:r, 0:1],
                                        op0=OP.mult, op1=OP.add)
                for kt in range(NT_K):
                    cs = slice(kt * 128, (kt + 1) * 128)
                    pt = psT.tile([128, 128], f32, tag="tp")
                    nc.tensor.transpose(pt[:, :r], xh[:r, cs], ident[:r, :r])
                    nc.scalar.copy(out=xhatT[kt][:, i * 128:i * 128 + r],
                                   in_=pt[:, :r])
                mus.append(mu)
                sigs.append(sig)

            # stage mu/sig to DRAM, read back broadcast over partitions
            # (for reconstructing X^T for the residual: X = xhat*sig + mu)
            with tc.tile_pool(name="dres", bufs=1, space="DRAM") as drp:
                mu_d = drp.tile([3 * 128, 1], f32, tag="mu_d")
                sig_d = drp.tile([3 * 128, 1], f32, tag="sig_d")
                for i in range(3):
                    r = rows[i]
                    nc.sync.dma_start(out=mu_d[i * 128:i * 128 + r, :],
                                      in_=mus[i][:r])
                    nc.sync.dma_start(out=sig_d[i * 128:i * 128 + r, :],
                                      in_=sigs[i][:r])
                mu_bc = persist.tile([128, LO], f32, tag="mu_bc")
                sig_bc = persist.tile([128, LO], f32, tag="sig_bc")
                own0 = WARM + K - 1
                for (dst, srcd) in ((mu_bc, mu_d), (sig_bc, sig_d)):
                    s_ap = srcd[own0:own0 + LO, :]
                    nc.sync.dma_start(
                        out=dst,
                        in_=bass.AP(tensor=s_ap.tensor, offset=s_ap.offset,
                                    ap=[[0, 128], [1, LO]]))

            p0x_cm.__exit__(None, None, None)
            p0_cm.__exit__(None, None, None)

            # ---- Phase A: mm1 + conv + silu -> X_main; gate per d-tile ----
            X_main = []
            X_gate = []
            gate_silus = []
            for dt in range(NT_D if "A" in phases else 0):
                w12 = wstream.tile([128, 2, NT_K * 128], bf16, tag="wst")
                nc.sync.dma_start(out=w12, in_=W12R_d[:, dt, :, :])
                w1t = w12[:, 0, :]
                ps = psA.tile([128, LC], f32, tag="mm")
                for kt in range(NT_K):
                    nc.tensor.matmul(ps, w1t[:, kt * 128:(kt + 1) * 128],
                                     xhatT[kt],
                                     start=(kt == 0), stop=(kt == NT_K - 1))
                # depthwise causal conv: 4 taps on DVE (Pool cannot read
                # PSUM on TRN2)
                acc = None
                for tap in range(K):
                    nxt = work.tile([128, LW], f32,
                                    tag=("caccA", "caccB")[tap % 2])
                    if acc is None:
                        nc.vector.tensor_scalar(
                            out=nxt, in0=ps[:, tap:tap + LW],
                            scalar1=convw_sb[dt][:, tap:tap + 1], scalar2=None,
                            op0=OP.mult)
                    else:
                        nc.vector.scalar_tensor_tensor(
                            out=nxt, in0=ps[:, tap:tap + LW],
                            scalar=convw_sb[dt][:, tap:tap + 1], in1=acc,
                            op0=OP.mult, op1=OP.add)
                    acc = nxt
                xm = persist.tile([128, LW], bf16, tag=f"xm{dt}")
                nc.scalar.activation(out=xm, in_=acc, func=AF.Silu,
                                     bias=cb2_sb[dt][:, 0:1], scale=1.0)
                X_main.append(xm)
                # gate = silu(xhat @ W2) for this d-tile (own L only)
                w2t = w12[:, 1, :]
                ps2 = psA.tile([128, LO], f32, tag="mm")
                for kt in range(NT_K):
                    nc.tensor.matmul(ps2, w2t[:, kt * 128:(kt + 1) * 128],
                                     xhatT[kt][:, WARM + K - 1:WARM + K - 1 + LO],
                                     start=(kt == 0), stop=(kt == NT_K - 1))
                xg = persist.tile([128, LO], bf16, tag=f"xg{dt}")
                si = nc.scalar.activation(out=xg, in_=ps2, func=AF.Silu,
                                          bias=c2_sb[dt][:, 0:1], scale=1.0)
                gate_silus.append(si)
                X_gate.append(xg)

            # ---- Phase B part 1: B/C rows of pp -> bci -> DRAM stage ----
            # (issued before the gate matmuls so the DRAM roundtrip latency
            # overlaps phase A2)
            wbt = wstream.tile([128, NT_D * 2 * N], bf16, tag="wbc")
            nc.sync.dma_start(out=wbt, in_=WbcR_d[:, :])
            psbc = psA.tile([2 * N, LW], f32, tag="mm")
            for kt in range(NT_D):
                nc.tensor.matmul(psbc, wbt[:, kt * 2 * N:(kt + 1) * 2 * N],
                                 X_main[kt],
                                 start=(kt == 0), stop=(kt == NT_D - 1))
            bc_raw = work.tile([2 * N, LW], f32, tag="caccA")
            nc.scalar.activation(out=bc_raw, in_=psbc, func=AF.Identity,
                                 bias=bbc_sb[:, 0:1], scale=1.0)
            bci = work.tile([2 * N, LW], bf16, tag="bci")
            nc.vector.scalar_tensor_tensor(out=bci, in0=bc_raw,
                                           scalar=invAv_sb[:, 0:1],
                                           in1=mask_sb, op0=OP.mult,
                                           op1=OP.mult)
            dstage_cm = tc.tile_pool(name="dstage", bufs=1, space="DRAM")
            dpool = dstage_cm.__enter__()
            bci_dram = dpool.tile([2 * N, LW], bf16, tag="bcid")
            nc.sync.dma_start(out=bci_dram, in_=bci)

            # ---- Phase B part 2: partition-broadcast reads ----
            Bm_bcI = persist.tile([128, N, LW], bf16, tag="BmbcI")
            Cm_bc = persist.tile([128, N, LO], bf16, tag="Cmbc")
            src_b = bci_dram[0:1, :]
            nc.sync.dma_start(
                out=Bm_bcI,
                in_=bass.AP(tensor=src_b.tensor, offset=src_b.offset,
                            ap=[[0, 128], [LW, N], [1, LW]]))
            src_c = bci_dram[N:N + 1, WARM:LW]
            nc.sync.dma_start(
                out=Cm_bc,
                in_=bass.AP(tensor=src_c.tensor, offset=src_c.offset,
                            ap=[[0, 128], [LW, N], [1, LO]]))
            dstage_cm.__exit__(None, None, None)

            # ---- Phase C: per d-tile: delta, a-powers, u, scan, y ----
            # down-projection accumulators live across the C loop
            # (two e8 accumulators packed per 2KB PSUM bank)
            psDacc = []
            for pb in range(NT_K // 2):
                dacc = psD.tile([128, 2, LO], f32, tag=f"dacc{pb}")
                psDacc.append(dacc[:, 0, :])
                psDacc.append(dacc[:, 1, :])
            y_gated = []
            first_c_act = None
            for dt in range(NT_D):
                wllt = wstream.tile([128, NT_D * 128], bf16, tag="wll")
                nc.sync.dma_start(out=wllt, in_=WllR_d[:, dt, :])
                ps = psA.tile([128, LW], f32, tag="mm")
                for kt in range(NT_D):
                    nc.tensor.matmul(ps, wllt[:, kt * 128:(kt + 1) * 128],
                                     X_main[kt],
                                     start=(kt == 0), stop=(kt == NT_D - 1))
                # softplus(x) = ln(exp(x) + 1); exp & ln share one ACT table
                e1 = work.tile([128, LW], f32, tag="caccA")
                e1i = nc.scalar.activation(out=e1, in_=ps, func=AF.Exp,
                                           bias=bd_sb[dt][:, 0:1], scale=1.0)
                if dt == 0:
                    from concourse.tile_rust import add_dep_helper
                    for si in gate_silus:
                        add_dep_helper(e1i.ins, si.ins, False,
                                       "ACT table-set phase ordering")
                    first_c_act = e1i
                delta = work.tile([128, LW], f32, tag="caccB")
                nc.scalar.activation(out=delta, in_=e1, func=AF.Ln,
                                     bias=1.0, scale=1.0)

                # decay powers a_n = E^(n+1): N_ACT_EXP direct exps on ACT,
                # the rest from one bf16 DVE multiply
                apow = nl3.tile([128, N, LW], bf16, tag="apow")
                for n in range(N_ACT_EXP):
                    nc.scalar.activation(out=apow[:, n, :], in_=delta,
                                         func=AF.Exp, bias=0.0,
                                         scale=A_sb[:, n:n + 1])
                if N_ACT_EXP < N:
                    lo = N_ACT_EXP - 8
                    nc.vector.tensor_tensor(
                        out=apow[:, N_ACT_EXP:N, :],
                        in0=apow[:, lo:8, :],
                        in1=bcast_n(apow[:, 7, :], N - N_ACT_EXP),
                        op=OP.mult)
                am1 = nl.tile([128, N, LW], bf16, tag="amh")
                nc.vector.tensor_scalar(out=am1, in0=apow, scalar1=-1.0,
                                        scalar2=None, op0=OP.add)
                # w and u: low n-half on DVE (bf16 2x), high half on Pool
                w_t = nl4.tile([128, N, LW], bf16, tag="wh")
                nc.vector.tensor_tensor(out=w_t[:, 0:8, :],
                                        in0=bcast_n(X_main[dt], 8),
                                        in1=Bm_bcI[:, 0:8, :], op=OP.mult)
                nc.gpsimd.tensor_tensor(out=w_t[:, 8:16, :],
                                        in0=bcast_n(X_main[dt], 8),
                                        in1=Bm_bcI[:, 8:16, :], op=OP.mult)
                u_t = nl4.tile([128, N, LW], bf16, tag="u")
                nc.vector.tensor_tensor(out=u_t[:, 0:8, :], in0=am1[:, 0:8, :],
                                        in1=w_t[:, 0:8, :], op=OP.mult)
                nc.gpsimd.tensor_tensor(out=u_t[:, 8:16, :],
                                        in0=am1[:, 8:16, :],
                                        in1=w_t[:, 8:16, :], op=OP.mult)
                # zero decay at each n-segment start: encodes h(start)=u
                nc.vector.memset(apow[:, :, 0:1], 0.0)
                h_t = nl.tile([128, N, LW], bf16, tag="amh")
                nc.vector.tensor_tensor_scan(
                    out=h_t.rearrange("p n l -> p (n l)"),
                    data0=apow.rearrange("p n l -> p (n l)"),
                    data1=u_t.rearrange("p n l -> p (n l)"),
                    initial=0.0, op0=OP.mult, op1=OP.add)
                hci_t = nl4.tile([128, N, LW], bf16, tag="wh")
                hci = hci_t[:, :, 0:LO]
                nc.vector.tensor_tensor(out=hci, in0=h_t[:, :, WARM:LW],
                                        in1=Cm_bc, op=OP.mult)
                # sum over n on the (mostly idle) PE: 16 accumulating
                # identity matmuls into PSUM
                psy = psY.tile([128, LO], f32, tag="ytree")
                for n in range(N):
                    nc.tensor.matmul(psy, identb, hci[:, n, :],
                                     start=(n == 0), stop=(n == N - 1),
                                     skip_group_check=True)
                yg = persist.tile([128, LO], bf16, tag=f"yg{dt}")
                nc.vector.tensor_tensor(out=yg, in0=psy, in1=X_gate[dt],
                                        op=OP.mult)
                y_gated.append(yg)
                # fold this d-tile into the down-projection accumulators
                wd_dt = wstream.tile([128, NT_K * 128], bf16, tag="wd")
                nc.sync.dma_start(out=wd_dt, in_=WdR_d[:, dt, :])
                for e8 in range(NT_K):
                    nc.tensor.matmul(psDacc[e8],
                                     wd_dt[:, e8 * 128:(e8 + 1) * 128],
                                     yg, start=(dt == 0), stop=(dt == NT_D - 1),
                                     skip_group_check=True)

            # ---- Phase D: bias + residual + store ----
            for e8 in range(NT_K):
                xrec = work.tile([128, LO], f32, tag="caccA")
                nc.gpsimd.tensor_tensor(out=xrec,
                                        in0=xhatT[e8]
                                        [:, WARM + K - 1:WARM + K - 1 + LO],
                                        in1=sig_bc, op=OP.mult)
                xrec2 = work.tile([128, LO], f32, tag="caccB")
                nc.vector.tensor_tensor(out=xrec2, in0=xrec, in1=mu_bc,
                                        op=OP.add)
                osb = work.tile([128, LO], f32, tag="cacc")
                nc.vector.scalar_tensor_tensor(
                    out=osb, in0=psDacc[e8], scalar=bdown_sb[e8][:, 0:1],
                    in1=xrec2, op0=OP.add, op1=OP.add)
                nc.sync.dma_start(out=Y_d[e8 * 128:(e8 + 1) * 128, :], in_=osb)

    nc.compile()
    return nc


def kernel(X, ln_g, ln_b, W_up1, conv_w, conv_b, W_ll, b_ll, A_log, W_up2,
           W_down, b_down):
    from concourse.bass_utils import run_bass_kernel_spmd
    import ml_dtypes

    f = np.float32
    bf = ml_dtypes.bfloat16
    X = np.asarray(X, f)
    A = -np.exp(np.asarray(A_log, f))
    assert np.allclose(A, A[0:1, :]), "kernel assumes A rows identical"
    c1 = (np.asarray(W_up1, f) @ np.asarray(ln_b, f)).astype(f)
    c2 = (np.asarray(W_up2, f) @ np.asarray(ln_b, f)).astype(f)
    cw = np.asarray(conv_w, f)[:, 0, :]                      # [D, K]
    cb2 = (np.asarray(conv_b, f) + c1 * cw.sum(1)).astype(f)

    # weight layouts: per-partition-contiguous slices for big DMA chunks
    W1g = (np.asarray(W_up1, f) * np.asarray(ln_g, f)[None, :])  # [D, DO]
    W2g = (np.asarray(W_up2, f) * np.asarray(ln_g, f)[None, :])
    Wll = np.asarray(W_ll, f)                                    # [2N+D, D]
    Wd = np.asarray(W_down, f)                                   # [DO, D]
    # W12R[p, dt, 0, kt*128+j] = W1g[dt*128+j, kt*128+p]; [.., 1, ..] = W2g
    W12R = np.stack([
        W1g.reshape(NT_D, 128, NT_K, 128).transpose(3, 0, 2, 1)
        .reshape(128, NT_D, NT_K * 128),
        W2g.reshape(NT_D, 128, NT_K, 128).transpose(3, 0, 2, 1)
        .reshape(128, NT_D, NT_K * 128)], axis=2).astype(bf)
    # WllR[p, dt, kt*128+j] = Wll[dt*128+j, kt*128+p]  (delta rows)
    WllR = np.ascontiguousarray(
        Wll[:D].reshape(NT_D, 128, NT_D, 128).transpose(3, 0, 2, 1)
        .reshape(128, NT_D, NT_D * 128)).astype(bf)
    # WbcR[p, kt*2N+c] = Wll[D+c, kt*128+p]  (B/C rows)
    WbcR = np.ascontiguousarray(
        Wll[D:].reshape(2 * N, NT_D, 128).transpose(2, 1, 0)
        .reshape(128, NT_D * 2 * N)).astype(bf)
    # WdR[p, dt, e8*128+j] = Wd[e8*128+j, dt*128+p]
    WdR = np.ascontiguousarray(
        Wd.reshape(NT_K, 128, NT_D, 128).transpose(3, 2, 0, 1)
        .reshape(128, NT_D, NT_K * 128)).astype(bf)

    # packed per-d-tile constants [128, NT_D, 7]
    CstD = np.empty((128, NT_D, 7), f)
    CstD[:, :, 0:4] = cw.reshape(NT_D, 128, K).transpose(1, 0, 2)
    CstD[:, :, 4] = cb2.reshape(NT_D, 128).T
    CstD[:, :, 5] = np.asarray(b_ll, f)[:D].reshape(NT_D, 128).T
    CstD[:, :, 6] = c2.reshape(NT_D, 128).T
    CstO = np.empty((128, 8 + N), f)
    CstO[:, 0:8] = np.asarray(b_down, f).reshape(NT_K, 128).T
    CstO[:, 8:] = np.tile(A[0:1, :], (128, 1))
    BbcI = np.stack(
        [np.asarray(b_ll, f)[D:],
         np.concatenate([1.0 / A[0], np.ones(N, f)]).astype(f)], axis=1)
    shared = {
        "W12R": np.ascontiguousarray(W12R), "WllR": WllR,
        "WbcR": WbcR, "WdR": WdR,
        "CstD": CstD, "CstO": CstO, "BbcI": np.ascontiguousarray(BbcI),
    }
    in_maps = []
    for c in range(NCORES):
        b, q = divmod(c, 4)
        l0 = q * LO
        lo_ext = l0 - (WARM + K - 1)
        xs = np.zeros((LC, D_OUTER), f)
        src0 = max(0, lo_ext)
        hi = min(l0 + LO + 1, L)
        xs[src0 - lo_ext:src0 - lo_ext + (hi - src0), :] = X[b, src0:hi, :]
        mask = np.ones((1, LW), f)
        if q == 0:
            mask[0, :WARM] = 0.0
        in_maps.append({"Xs": xs, "mask": mask, **shared})

    nc = _build_program()
    res = run_bass_kernel_spmd(nc, in_maps, core_ids=list(range(NCORES)))
    global last_result
    last_result = res

    out = np.empty((B_SZ, L, D_OUTER), f)
    for c in range(NCORES):
        b, q = divmod(c, 4)
        out[b, q * LO:(q + 1) * LO, :] = res.results[c]["Y"].T
    return out
